# revision 26
# baseline (speedup 1.0000x reference)
"""Dark channel prior loss on 8 trn2 NeuronCores.

Reference computes: reflect-pad H/W by 7, min over (C, H, W) per image,
mean over batch. Reflect padding only duplicates interior values, so it
cannot change a min — the loss is exactly mean_b(min_chw(x[b])).

Data-parallel: 4 images per core. Host downcasts f32 -> bf16 with
round-to-nearest (min is exact in any precision, so the only error is
the initial rounding: measured 7e-4 on the final loss vs the 2e-2
gate), halving the HBM stream to 6.29 MB/core (~15 us at the measured
~420 GB/s/core — the per-core DMA ceiling; SWDGE and HWDGE share the
same 16 SDMA queues, so splitting paths does not add bandwidth).

Reduction (all on the DVE — GpSimd cannot run TensorTensor and the
DMA CCE only supports add): hardware-measured rates are
  tensor_reduce  1.06 ns/elem (no perf modes)
  tensor_tensor  0.54 ns/out  (2x_1p confirmed for packed bf16)
  tensor_tensor_scan  4.4 ns/out (cost model says 1x — HW is 4x worse)
so everything is built from tensor_tensor min folds: per image a
[128, 1536] accumulator folds the arriving chunks in place
(acc = min(acc, piece) — each op consumes 1536 fresh columns per
830 ns, just ahead of the 1.9 us/chunk DMA arrival rate), then a
halving tree folds acc to [128, PW=768] which is DMA'd out per image
(images 0-2's outs fly during the stream); the host finishes
min-over-PW-cols/partitions per image and the batch mean. Image 3's
tail chunks stream as 1536/768/768-col pieces and its tree runs
BEFORE the sub-PW pieces (they fold straight into the final PW
window), so the only exposed post-stream work is one 415 ns fold +
the out-DMA issue/flight.

Raw bacc kernel (no TileContext): GpSimd issues all chunk DMAs
back-to-back (hoisted before the init barrier so the HBM stream starts
at launch; the barrier's Pool DRAIN is defused to a NOP carrying the
same semaphore protocol, since a real GpSimd DRAIN waits for all
outstanding SWDGE DMAs). One completion sem per chunk DMA, waited to
exactly 16 (one inc per SDMA engine — a shared cumulative counter is
unsound across interleaved per-engine increments). The partial is
DMA'd out with no completion wait (the runtime's end-of-program Pool
DRAIN already blocks on SWDGE queue completion) while one range-clear
resets the sems for repeat executions.
"""

import ml_dtypes
import numpy as np

import concourse.bass as bass  # noqa: F401
from concourse import bacc, mybir
from concourse.bass_utils import run_bass_kernel_spmd


def _install_ntff_hook():
    """This image's antenv lacks axon_hooks, so a traced run (trace=True or
    BASS_TRACE=1) would crash inside run_bass_kernel_spmd on the import.
    Synthesize the module around trn_boot's ctypes NTFF hook; degrade
    silently if any piece is missing."""
    import sys
    import types

    if "antenv.axon_hooks" in sys.modules:
        return
    try:
        sys.path.insert(0, "/root/.axon_site")
        from trn_agent_boot.trn_boot import _ntff_profile_via_ctypes

        hook = _ntff_profile_via_ctypes("/opt/axon/libaxon_pjrt.so")
        mod = types.ModuleType("antenv.axon_hooks")
        mod._hook = hook
        mod.get_axon_ntff_profile_hook = lambda: mod._hook
        mod.set_axon_ntff_profile_hook = lambda h: setattr(mod, "_hook", h)
        sys.modules["antenv.axon_hooks"] = mod
    except Exception:
        pass


_install_ntff_hook()

N_CORES = 8
B = 32
PER_CORE = B // N_CORES  # 4 images per core
P = 128
F = 3 * 512 * 512 // P  # 6144 elements per partition per image
TOTAL = PER_CORE * F  # 24576 columns of [128, .] per core

AW = 1536  # accumulator width per image
PW = 768  # partial width for the LAST image (tree stops here: its last
# tree op would sit on the critical-path tail)
# Images 0-2 fold one level further (384): their out-DMAs fly mid-stream
# and steal SDMA engine time from the input stream, so halving their
# bytes trims the interleave; the extra 269 ns tree op per image hides
# in mid-stream DVE slack. The host mins only each image's live columns.
PWS = [384, 384, 384, PW]

# (image, offset, width) per input DMA, in stream order. Images 0-2
# stream as two 3072-col chunks; image 3's tail is split 3072/1536/768/768
# so the post-stream work is one small fold + the final tree.
CHUNKS = []
for b in range(3):
    CHUNKS += [(b, 0, 3072), (b, 3072, 3072)]
CHUNKS += [(3, 0, 3072), (3, 3072, 1536), (3, 4608, 768), (3, 5376, 768)]
assert all(off + w <= F for _, off, w in CHUNKS)
assert sum(w for _, _, w in CHUNKS) == TOTAL

_nc_cache = None


def _build_nc(optimize: bool = True):
    nc = bacc.Bacc(trn_type="TRN2", debug=False, num_devices=N_CORES)
    x = nc.dram_tensor("x", [PER_CORE, P, F], mybir.dt.bfloat16, kind="ExternalInput")
    out = nc.dram_tensor(
        "out", [P, PER_CORE, PW], mybir.dt.bfloat16, kind="ExternalOutput"
    )
    x_ap = x.ap()

    nchunk = len(CHUNKS)
    chunk_sems = [nc.alloc_semaphore(f"dma_done_{c}") for c in range(nchunk)]
    red_sem = nc.alloc_semaphore("red_done")
    out_sem = nc.alloc_semaphore("out_done")
    buf = nc.alloc_sbuf_tensor("buf", [P, TOTAL], mybir.dt.bfloat16)
    acc = nc.alloc_sbuf_tensor("acc", [P, PER_CORE, AW], mybir.dt.bfloat16)

    # All chunk DMAs issue from GpSimd SWDGE. (Measured: moving chunks to
    # Sync/Scalar's HWDGE path is a consistent LOSS — the HWDGE/SWDGE
    # interleave on the shared 16 SDMA queues services each DMA slower,
    # and HWDGE adds no bandwidth since both paths feed the same queues.)
    load_by_engine = {}
    for c, (b, off, w) in enumerate(CHUNKS):
        s = b * F + off
        # The FIRST chunk issues from Sync (SP enters main ~0.8 us before
        # GpSimd's sequencer reaches its first DIRECT2D), starting the
        # stream earlier; everything else stays on GpSimd SWDGE.
        eng = nc.sync if c == 0 else nc.gpsimd
        bi = eng.dma_start(
            buf.ap()[:, s : s + w], x_ap[b][:, off : off + w]
        ).then_inc(chunk_sems[c], 16)
        load_by_engine.setdefault(eng, []).append(bi.ins)

    # Per-image fold + tree on the DVE, interleaved with chunk arrivals.
    # Same-engine program order serializes the ops; each op that first
    # touches a chunk waits on that chunk's DMA sem. All tensor_tensor
    # operands are packed bf16 (2x_1p: 0.54 ns/out elem).
    mn = mybir.AluOpType.min
    v = nc.vector
    for b in range(PER_CORE):
        a = acc.ap()[:, b, :]  # [128, AW] contiguous
        pieces = [c for c, (bb, _, _) in enumerate(CHUNKS) if bb == b]
        first = True
        wacc = AW  # current live accumulator width
        for ci, c in enumerate(pieces):
            _, off, w = CHUNKS[c]
            s = b * F + off
            if first:
                # acc = min(chunk first half, chunk second half)
                assert w == 2 * AW
                v.tensor_tensor(
                    out=a,
                    in0=buf.ap()[:, s : s + AW],
                    in1=buf.ap()[:, s + AW : s + w],
                    op=mn,
                )._wait_ge(chunk_sems[c], 16)
                first = False
                continue
            # Once only sub-PW pieces remain, run the AW->PW tree NOW so
            # it is off the post-stream tail; the remaining pieces fold
            # straight into the final PW window.
            if wacc > PWS[b] and all(CHUNKS[p][2] <= PWS[b] for p in pieces[ci:]):
                wcur = wacc
                while wcur > PWS[b]:
                    h = wcur // 2
                    v.tensor_tensor(
                        out=acc.ap()[:, b, :h],
                        in0=acc.ap()[:, b, :h],
                        in1=acc.ap()[:, b, h:wcur],
                        op=mn,
                    )
                    wcur = h
                wacc = PWS[b]
            # fold the piece into acc in wacc-wide strips (in-place: out
            # aliases in0 element-for-element, which the DVE pipeline
            # handles; strips narrower than wacc fold onto acc's prefix)
            done = 0
            while done < w:
                sw = min(wacc, w - done)
                bi = v.tensor_tensor(
                    out=acc.ap()[:, b, :sw],
                    in0=acc.ap()[:, b, :sw],
                    in1=buf.ap()[:, s + done : s + done + sw],
                    op=mn,
                )
                if done == 0:
                    bi._wait_ge(chunk_sems[c], 16)
                done += sw
        # halving tree down to PWS[b] (no-op if the early tree already
        # ran); acc[:, b, :PWS[b]] is this image's partial
        while wacc > PWS[b]:
            h = wacc // 2
            bi = v.tensor_tensor(
                out=acc.ap()[:, b, :h],
                in0=acc.ap()[:, b, :h],
                in1=acc.ap()[:, b, h:wacc],
                op=mn,
            )
            wacc = h
        bi.then_inc(red_sem, 1)

    # Per-image out DMAs: images 0-2's partials fly while the stream is
    # still running; only image 3's is in the tail. DVE runs in program
    # order, so red_sem == b+1 exactly when image b's tree is done.
    out_bi = None
    for b in range(PER_CORE):
        out_bi = nc.gpsimd.dma_start(
            out.ap()[:, b, : PWS[b]], acc.ap()[:, b, : PWS[b]]
        )._wait_ge(red_sem, b + 1).then_inc(out_sem, 16)
    # Reset kernel sems (one contiguous range clear) so a repeat execution
    # of the same NEFF starts clean. Nothing waits on out_sem (the DMA
    # lowering just needs an update target): the runtime's end-of-program
    # Pool DRAIN blocks until the SWDGE queue has fully completed, which
    # guarantees the output landed before the NEFF execution retires.
    assert out_sem.num == chunk_sems[0].num + nchunk + 1
    nc.gpsimd.sem_clear(range(chunk_sems[0].num, out_sem.num + 1))

    if optimize:
        # Hoist the load DMAs to right after GpSimd's register preamble
        # (same splice point bacc uses for its kernel-barrier collective)
        # so the HBM stream starts before the init barrier. Nothing before
        # the barrier reads buf, and dma_sem was reset by the previous
        # execution's tail. Then defuse the init barrier's Pool DRAINs:
        # a GpSimd DRAIN waits for ALL outstanding SWDGE DMAs, which
        # would serialize the hoisted stream; a NOP carrying the same
        # semaphore protocol preserves the barrier — every data
        # dependency rides an explicit sem. Applied to a scratch list so
        # a failure leaves the (still-correct, ~3us slower) unhoisted
        # layout intact.
        try:
            entry = nc.main_func.blocks[0]
            insts = list(entry.instructions)
            for eng, load_insts in load_by_engine.items():
                assert eng.preamble_end is not None
                for inst in load_insts:
                    insts.remove(inst)
                idx = insts.index(eng.preamble_end) + 1
                insts[idx:idx] = load_insts

            issue_engines = {eng.engine for eng in load_by_engine}
            for pos, inst in enumerate(insts):
                if inst is out_bi.ins:
                    break
                if isinstance(inst, mybir.InstDrain) and inst.engine in issue_engines:
                    nop = mybir.InstNoOp(
                        name=nc.get_next_instruction_name(), ins=[], outs=[]
                    )
                    nop.engine = inst.engine
                    nop.sync_info = inst.sync_info
                    nc.register_instruction(nop)
                    insts[pos] = nop

            entry.instructions[:] = insts
        except Exception:
            return _build_nc(optimize=False)

    nc.finalize()
    return nc


def _run_spmd(x: np.ndarray, **kwargs):
    """x: full [32,3,512,512] f32. Returns BassKernelResults."""
    global _nc_cache
    if _nc_cache is None:
        _nc_cache = _build_nc()
    shards = (
        np.ascontiguousarray(x)
        .astype(ml_dtypes.bfloat16)
        .reshape(N_CORES, PER_CORE, P, F)
    )
    in_maps = [{"x": shards[i]} for i in range(N_CORES)]
    return run_bass_kernel_spmd(
        _nc_cache, in_maps, core_ids=list(range(N_CORES)), **kwargs
    )


def kernel(input_image: np.ndarray) -> np.ndarray:
    x = np.asarray(input_image, dtype=np.float32)
    res = _run_spmd(x)
    # [8, 128, PER_CORE, PW] -> per-image mins -> mean over 32 images
    partials = np.stack(
        [np.asarray(r["out"]).astype(np.float32) for r in res.results]
    )  # [8, P, PER_CORE, PW]; image b's live columns are [:PWS[b]]
    per_image = np.stack(
        [partials[:, :, b, : PWS[b]].min(axis=(1, 2)) for b in range(PER_CORE)],
        axis=1,
    )  # [8, PER_CORE]
    return np.asarray(per_image.mean(), dtype=np.float32)
